# revision 1
# baseline (speedup 1.0000x reference)
"""Multi-head causal self-attention (B=2, S=2048, D=1024, H=16) on 8 NeuronCores.

Sharding: core c handles batch b = c // 4 and heads 4j..4j+3 where j = c % 4
(tensor-parallel over heads within a 4-core group, data-parallel over batch).

Structure (vs the original baseline):
  * all matmul operands are bf16 (host-converted); accumulation stays f32 in
    PSUM.  Halves DMA traffic and removes the f32r small-free-dim penalty.
  * emission interleaves phases so the Act engine (softmax exp) starts ~12us
    in instead of ~58us: first K/Q block + first V tiles, then pair-0
    attention with remaining V/K/Q projections injected into the attention
    k-loops, then pair-1 attention with pair-1 K/Q and the output projection
    injected likewise.
  * scores for diagonal k-tiles skip the fully-masked columns in the S^T
    matmul, the exp, and the PV matmul.
  * normalized ctx^T is AllGathered across the 4-core group in three pieces
    (pair 0 full; pair 1 q<1536; pair 1 tail) and each core computes its
    256-column slice of the output projection, overlapped with attention.

Per-head softmax denominator comes from an appended ones-column in V (row DH
of the ctx PSUM tile).  Heads are processed in pairs sharing 128 partitions
(rows 0-63 = even head, 64-127 = odd head of the pair).
"""

import math

import numpy as np
import ml_dtypes

import concourse.tile as tile
from concourse import bacc, mybir
from concourse.bass_utils import run_bass_kernel_spmd

B, S, D, H, DH = 2, 2048, 1024, 16, 64
NCORES = 8
GROUP = 4          # cores per batch group
HPC = 4            # heads per core
FPC = HPC * DH     # 256 features per core
QB = 512           # q block width
KT = 128           # k tile height (partition dim)
SCALE = 1.0 / math.sqrt(S)

F32 = mybir.dt.float32
BF16 = mybir.dt.bfloat16
EXP = mybir.ActivationFunctionType.Exp
BF = ml_dtypes.bfloat16


def build_program(sim_collective=False, reps=1):
    """sim_collective=True replaces the AllGathers with equivalent-volume
    local DMA traffic so the (single-core) TimelineSim cost model can run.
    reps>1 repeats the whole body inside one NEFF (for slope timing)."""
    nc = bacc.Bacc(
        "TRN2",
        target_bir_lowering=False,
        debug=False,
        num_devices=NCORES,
    )

    xT = nc.dram_tensor("xT", [D, S], BF16, kind="ExternalInput").ap()
    wq = nc.dram_tensor("wq", [D, FPC], BF16, kind="ExternalInput").ap()
    wk = nc.dram_tensor("wk", [D, FPC], BF16, kind="ExternalInput").ap()
    wv = nc.dram_tensor("wv", [D, FPC], BF16, kind="ExternalInput").ap()
    # wo columns for this core, rows permuted to the gathered ctx^T order
    wo = nc.dram_tensor("wo", [D, FPC], BF16, kind="ExternalInput").ap()
    bo = nc.dram_tensor("bo", [1, FPC], F32, kind="ExternalInput").ap()
    tri = nc.dram_tensor("tri", [KT, 2 * KT], BF16, kind="ExternalInput").ap()
    ones = nc.dram_tensor("ones", [128, 16 * HPC], BF16, kind="ExternalInput").ap()
    out = nc.dram_tensor("out", [S, FPC], F32, kind="ExternalOutput").ap()

    with tile.TileContext(nc) as tc:
      for _rep in range(reps):
        with (
            tc.tile_pool(name="cpool", bufs=1) as cpool,
            tc.tile_pool(name="qkvp", bufs=1) as qkvp,
            tc.tile_pool(name="dpool", bufs=1, space="DRAM") as dpool,
        ):
            # ---- persistent SBUF tensors ---------------------------------
            wq_sb = cpool.tile([128, 8, FPC], BF16)
            wk_sb = cpool.tile([128, 8, FPC], BF16)
            wv_sb = cpool.tile([128, 8, FPC], BF16)
            wo_sb = cpool.tile([128, 8, FPC], BF16)
            xt_sb = cpool.tile([128, 8, S], BF16)
            tri_sb = cpool.tile([KT, 2, KT], BF16)
            bias_bc = cpool.tile([128, FPC], F32)

            qT_sb = qkvp.tile([128, 2, S], BF16)   # [dh-pair, pair, seq]
            kT_sb = qkvp.tile([128, 2, S], BF16)
            v_sb = qkvp.tile([128, 16, HPC * (DH + 1)], BF16)
            v4 = v_sb.rearrange("p s (h e) -> p s h e", e=DH + 1)

            cc_in0 = dpool.tile([2 * DH, S], BF16)
            cc_in1a = dpool.tile([2 * DH, 3 * S // 4], BF16)
            cc_in1b = dpool.tile([2 * DH, S // 4], BF16)
            cc_out0 = dpool.tile([GROUP * 2 * DH, S], BF16)
            cc_out1a = dpool.tile([GROUP * 2 * DH, 3 * S // 4], BF16)
            cc_out1b = dpool.tile([GROUP * 2 * DH, S // 4], BF16)

            # ---- DMA loads, in consumption order -------------------------
            wq_d = wq.rearrange("(t p) f -> p t f", p=128)
            wk_d = wk.rearrange("(t p) f -> p t f", p=128)
            xt_dram_a = xT.rearrange("(t p) m -> p t m", p=128)
            nc.sync.dma_start(xt_sb[:, :, 0:QB], xt_dram_a[:, :, 0:QB])
            nc.sync.dma_start(wk_sb[:, :, 0:128], wk_d[:, :, 0:128])
            nc.sync.dma_start(wq_sb[:, :, 0:128], wq_d[:, :, 0:128])
            nc.sync.dma_start(wv_sb[:], wv.rearrange("(t p) f -> p t f", p=128))
            nc.sync.dma_start(tri_sb[:], tri.rearrange("p (h q) -> p h q", q=KT))
            nc.sync.dma_start(
                v4[:, :, :, DH], ones.rearrange("p (s h) -> p s h", h=HPC)
            )
            for c in range(1, 4):
                cs = slice(c * QB, (c + 1) * QB)
                nc.sync.dma_start(xt_sb[:, :, cs], xt_dram_a[:, :, cs])
            nc.sync.dma_start(wk_sb[:, :, 128:256], wk_d[:, :, 128:256])
            nc.sync.dma_start(wq_sb[:, :, 128:256], wq_d[:, :, 128:256])
            bo_sb = cpool.tile([1, FPC], F32)
            nc.sync.dma_start(bo_sb[:], bo)
            nc.gpsimd.partition_broadcast(bias_bc[:], bo_sb[:])
            nc.sync.dma_start(wo_sb[:], wo.rearrange("(t p) f -> p t f", p=128))

            # ---- pools ----------------------------------------------------
            # PSUM (8 banks): pj 2x[128,512]f32 (2, right) + st 2x[128,2,512]
            # f32 (4) + ctx 2x[65,512]f32 (2); era3 swaps pj for op tiles.
            attps = tc.alloc_tile_pool(name="attps", bufs=1, space="PSUM")
            pjp = tc.alloc_tile_pool(name="pjp", bufs=1, space="PSUM",
                                     side="right")
            attp = tc.alloc_tile_pool(name="attp", bufs=8)
            nrmp = tc.alloc_tile_pool(name="nrmp", bufs=4)
            ogp = tc.alloc_tile_pool(name="ogp", bufs=1)
            obp = tc.alloc_tile_pool(name="obp", bufs=3)

            ctxg = [ogp.tile([128, S], BF16, name=f"ctxg{f}", tag=f"ctxg{f}")
                    for f in range(8)]
            ccg0 = cc_out0.rearrange("(f p) q -> f p q", p=128)
            ccg1a = cc_out1a.rearrange("(f p) q -> f p q", p=128)
            ccg1b = cc_out1b.rearrange("(f p) q -> f p q", p=128)

            def emit_v(s):
                ps = pjp.tile([128, FPC], F32, tag="pj", bufs=2,
                              name=f"pv_{s}")
                for t in range(8):
                    nc.tensor.matmul(
                        ps[:],
                        xt_sb[:, t, s * 128:(s + 1) * 128],
                        wv_sb[:, t],
                        start=(t == 0),
                        stop=(t == 7),
                    )
                nc.vector.tensor_copy(
                    v4[:, s, :, 0:DH],
                    ps.rearrange("p (h e) -> p h e", e=DH),
                )

            def _kq_mms(ps, f, w_sb, qb, ts):
                for t in ts:
                    nc.tensor.matmul(
                        ps[:],
                        w_sb[:, t, f * 128:(f + 1) * 128],
                        xt_sb[:, t, qb * QB:(qb + 1) * QB],
                        start=(t == 0),
                        stop=(t == 7),
                    )

            def emit_kq(f, w_sb, dst, qb):
                ps = pjp.tile([128, QB], F32, tag="pj", bufs=2,
                              name=f"pkq_{f}_{qb}_{0 if w_sb is wk_sb else 1}")
                _kq_mms(ps, f, w_sb, qb, range(8))
                nc.vector.tensor_copy(dst[:, f, qb * QB:(qb + 1) * QB], ps[:])

            def kq_halves(f, which, qb):
                w_sb, dst = (wk_sb, kT_sb) if which == 0 else (wq_sb, qT_sb)
                return [lambda: emit_kq(f, w_sb, dst, qb), None]

            # out-proj tile: 128 q rows x this core's 256 out columns
            def emit_op(s, oph):
                ps = oph["p"].tile([128, FPC], F32, tag="op", bufs=2,
                                   name=f"op_{s}")
                for f in range(8):
                    nc.tensor.matmul(
                        ps[:],
                        ctxg[f][:, s * 128:(s + 1) * 128],
                        wo_sb[:, f],
                        start=(f == 0),
                        stop=(f == 7),
                    )
                ot = obp.tile([128, FPC], F32, tag="ot", name=f"ot_{s}")
                nc.vector.tensor_add(ot[:], ps[:], bias_bc[:])
                nc.sync.dma_start(out[s * 128:(s + 1) * 128, :], ot[:])

            def gather(cin, cout, dsts):
                if sim_collective:
                    for g in range(GROUP):
                        nc.sync.dma_start(
                            cout[g * 2 * DH:(g + 1) * 2 * DH, :], cin[:],
                        )
                else:
                    nc.gpsimd.collective_compute(
                        "AllGather",
                        mybir.AluOpType.bypass,
                        replica_groups=[[0, 1, 2, 3], [4, 5, 6, 7]],
                        ins=[cin.opt()],
                        outs=[cout.opt()],
                    )
                for dst, src in dsts:
                    nc.sync.dma_start(dst, src)

            def attention_block(pair, qb, inject=()):
                inject = list(inject)
                h0, h1 = 2 * pair, 2 * pair + 1
                nk = 4 * (qb + 1)
                q0 = qb * QB
                ctx0 = attps.tile([DH + 1, QB], F32, tag="ctx", bufs=2,
                                  name=f"ctx0_{pair}_{qb}")
                ctx1 = attps.tile([DH + 1, QB], F32, tag="ctx", bufs=2,
                                  name=f"ctx1_{pair}_{qb}")
                sts = [None] * nk
                pts = [None] * nk

                def emit_s(ki):
                    ks = slice(ki * KT, (ki + 1) * KT)
                    off = max(ki * KT - q0, 0)
                    qs = slice(q0 + off, q0 + QB)
                    st = attps.tile([128, 2, QB], F32, tag="st", bufs=2,
                                    name=f"st_{pair}_{qb}_{ki}")
                    nc.tensor.matmul(
                        st[:, 0, off:], kT_sb[0:64, pair, ks],
                        qT_sb[0:64, pair, qs], start=True, stop=True,
                    )
                    nc.tensor.matmul(
                        st[:, 1, off:], kT_sb[64:128, pair, ks],
                        qT_sb[64:128, pair, qs], start=True, stop=True,
                    )
                    sts[ki] = st

                def emit_exp(ki):
                    off = max(ki * KT - q0, 0)
                    pt = attp.tile([128, 2, QB], BF16, tag="pt",
                                   name=f"pt_{pair}_{qb}_{ki}")
                    nc.scalar.activation(
                        pt[:, :, off:], sts[ki][:, :, off:], EXP, scale=SCALE,
                    )
                    if ki * KT - q0 >= 0:
                        nc.vector.tensor_mul(
                            pt[:, :, off:off + KT],
                            pt[:, :, off:off + KT],
                            tri_sb[:],
                        )
                    pts[ki] = pt

                def emit_pv(ki):
                    pt = pts[ki]
                    off = max(ki * KT - q0, 0)
                    nc.tensor.matmul(
                        ctx0[:, off:], v4[:, ki, h0], pt[:, 0, off:],
                        start=(ki == 0), stop=(ki == nk - 1),
                    )
                    nc.tensor.matmul(
                        ctx1[:, off:], v4[:, ki, h1], pt[:, 1, off:],
                        start=(ki == 0), stop=(ki == nk - 1),
                    )

                emit_s(0)
                emit_exp(0)
                for ki in range(nk):
                    if ki + 1 < nk:
                        emit_s(ki + 1)
                    emit_pv(ki)
                    if ki + 1 < nk:
                        emit_exp(ki + 1)
                    if ki >= 1 and inject:
                        fn = inject.pop(0)
                        if fn is not None:
                            fn()
                for fn in inject:
                    if fn is not None:
                        fn()

                # normalize and store ctx^T (bf16) to the gather input
                for h, ctx in ((h0, ctx0), (h1, ctx1)):
                    rc = nrmp.tile([1, QB], F32, tag="rc",
                                   name=f"rc_{pair}_{qb}_{h}")
                    nc.vector.reciprocal(rc[:], ctx[DH:DH + 1, :])
                    bc = nrmp.tile([64, QB], F32, tag="bc",
                                   name=f"bc_{pair}_{qb}_{h}")
                    nc.gpsimd.partition_broadcast(bc[:], rc[:])
                    cn = nrmp.tile([64, QB], BF16, tag="cn",
                                   name=f"cn_{pair}_{qb}_{h}")
                    nc.vector.tensor_mul(cn[:], ctx[0:DH, :], bc[:])
                    row = slice((h % 2) * DH, (h % 2 + 1) * DH)
                    qs = slice(q0, q0 + QB)
                    if pair == 0:
                        nc.sync.dma_start(cc_in0[row, qs], cn[:])
                    elif qb < 3:
                        nc.sync.dma_start(cc_in1a[row, qs], cn[:])
                    else:
                        nc.sync.dma_start(cc_in1b[row, :], cn[:])

            # ---- era 1: first K/Q block, then first V tiles --------------
            emit_kq(0, wk_sb, kT_sb, 0)
            emit_kq(0, wq_sb, qT_sb, 0)
            for s in range(4):
                emit_v(s)

            # ---- era 2: pair-0 attention, projections injected -----------
            K, Q = 0, 1

            def kq(f, which, qb):
                w, d = (wk_sb, kT_sb) if which == K else (wq_sb, qT_sb)
                return lambda: emit_kq(f, w, d, qb)

            vj = lambda s: (lambda: emit_v(s))
            attention_block(0, 0, [vj(4), vj(5)])
            for fn in (vj(6), vj(7), kq(0, K, 1), kq(0, Q, 1)):
                fn()
            attention_block(0, 1,
                            [vj(8), vj(9), vj(10), vj(11)]
                            + kq_halves(0, K, 2) + kq_halves(0, Q, 2))
            attention_block(0, 2,
                            [vj(12), vj(13), vj(14), vj(15)]
                            + kq_halves(0, K, 3) + kq_halves(0, Q, 3))
            attention_block(0, 3,
                            kq_halves(1, K, 0) + kq_halves(1, Q, 0))
            # pair-0 ctx complete: gather it and load the f0-3 ctx^T tiles
            gather(cc_in0, cc_out0,
                   [(ctxg[f][:], ccg0[f]) for f in range(4)])

            # ---- era 3: pair-1 attention + out-proj interleaved ----------
            oph = {}
            attention_block(1, 0, kq_halves(1, K, 1) + kq_halves(1, Q, 1))
            attention_block(1, 1,
                            kq_halves(1, K, 2) + kq_halves(1, Q, 2)
                            + kq_halves(1, K, 3) + kq_halves(1, Q, 3))
            pjp.release()
            oph["p"] = tc.alloc_tile_pool(name="opp", bufs=1, space="PSUM",
                                          side="right")
            attention_block(1, 2)
            # pair-1 q<1536 gathered; f4-7 tiles for those columns load now
            gather(cc_in1a, cc_out1a,
                   [(ctxg[4 + f][:, 0:3 * S // 4], ccg1a[f])
                    for f in range(4)])
            attention_block(1, 3,
                            [(lambda s=s: emit_op(s, oph))
                             for s in range(12)])
            gather(cc_in1b, cc_out1b,
                   [(ctxg[4 + f][:, 3 * S // 4:], ccg1b[f])
                    for f in range(4)])
            for s in range(12, 16):
                emit_op(s, oph)

            oph["p"].release()
            obp.release()
            ogp.release()
            nrmp.release()
            attp.release()
            attps.release()

    nc.compile()
    return nc


_PROGRAM = None


def _get_program():
    global _PROGRAM
    if _PROGRAM is None:
        _PROGRAM = build_program()
    return _PROGRAM


def _make_tri():
    # tri[i, j] = 1 where key-offset i <= query-offset j (allowed); two
    # copies along the free dim serve the two heads of a fused pair tile
    i = np.arange(KT)[:, None]
    j = np.arange(KT)[None, :]
    t = (i <= j).astype(np.float32)
    return np.concatenate([t, t], axis=1)


def make_in_maps(x, Wq, Wk, Wv, Wo, bo):
    tri_arr = _make_tri().astype(BF)
    ones_arr = np.ones((128, 16 * HPC), BF)
    xTs = [np.ascontiguousarray(x[b].T).astype(BF) for b in range(B)]
    # Wo rows permuted to match the gathered ctx^T feature order:
    # gather0 rows = (rank j, heads 4j+0, 4j+1), gather1 = (rank j, 4j+2, 4j+3)
    perm = [4 * j + p for g in range(2) for j in range(GROUP)
            for p in (2 * g, 2 * g + 1)]
    Wo_perm = Wo.reshape(H, DH, D)[perm].reshape(D, D)
    in_maps = []
    for c in range(NCORES):
        b, j = divmod(c, GROUP)
        cols = slice(FPC * j, FPC * (j + 1))
        in_maps.append({
            "xT": xTs[b],
            "wq": np.ascontiguousarray(Wq[:, cols]).astype(BF),
            "wk": np.ascontiguousarray(Wk[:, cols]).astype(BF),
            "wv": np.ascontiguousarray(Wv[:, cols]).astype(BF),
            "wo": np.ascontiguousarray(Wo_perm[:, cols]).astype(BF),
            "bo": np.ascontiguousarray(bo[cols][None, :]).astype(np.float32),
            "tri": tri_arr,
            "ones": ones_arr,
        })
    return in_maps


def kernel(x, Wq, Wk, Wv, Wo, bo):
    x = np.ascontiguousarray(np.asarray(x, np.float32))
    Wq = np.asarray(Wq, np.float32)
    Wk = np.asarray(Wk, np.float32)
    Wv = np.asarray(Wv, np.float32)
    Wo = np.asarray(Wo, np.float32)
    bo = np.asarray(bo, np.float32)

    in_maps = make_in_maps(x, Wq, Wk, Wv, Wo, bo)
    nc = _get_program()
    results = run_bass_kernel_spmd(nc, in_maps, list(range(NCORES))).results

    out = np.empty((B, S, D), np.float32)
    for c in range(NCORES):
        b, j = divmod(c, GROUP)
        out[b, :, FPC * j:FPC * (j + 1)] = np.asarray(results[c]["out"],
                                                      np.float32)
    return out

